# revision 29
# baseline (speedup 1.0000x reference)
"""Multi-head attention (B=2, S=2048, H=1024, 16 heads) on 8 TRN2 NeuronCores.

Sharding: data parallel on batch (2) x tensor parallel on heads (4 heads/core,
Megatron column-split qkv, row-split wo). Host pre-transposes x/y, pre-scales
wq by dh^-0.5, and sum-reduces the 4 partial outputs per batch element.

Per-core kernel (v2): the ScalarE exp stream (1 elem/cycle/lane) paces the
steady state, so the wins are at the edges:
  - startup: a dummy exp preloads the ACT table at t~0; inputs arrive as
    split DMAs on two queues ordered so the minimal prefix (K[j4=0], Q[J=0],
    V[j4=0]) starts as soon as its bytes land; the remaining projection
    groups are woven into the early attention steps (ordered deque, budgeted
    emission; order respects each group's consumption deadline).
  - steady state: row-tiled (2-head packed) QK^T -> psl psum [128,1024] ->
    one exp per key block -> PV matmul with fused ones*exp(bias) denominator
    column (M=65) -> reciprocal + gpsimd partition-broadcast normalize ->
    pair-stacked bf16 ctx -> deferred output projection woven into later
    steps.
  - tail: the final q-block's output projection is emitted directly with
    its DMAs spread across queues.
"""
import sys
sys.path.insert(0, '/opt/trn_rl_repo')
import bisect
from collections import deque
from contextlib import ExitStack

import numpy as np
import ml_dtypes

import concourse.bacc as bacc
import concourse.tile as tile
from concourse import mybir
from concourse import bass_utils

B, S, H, NH = 2, 2048, 1024, 16
DH = H // NH            # 64
NCORES = 8
HPC = NH // (NCORES // B)   # 4 heads per core
C = HPC * DH            # 256 projected cols per core
KT_H = H // 128         # 8 contraction tiles over H
SK = S // 128           # 16 s-subtiles
JBLK = 512
NJ = S // JBLK          # 4 q-blocks
F32 = mybir.dt.float32
BF16 = mybir.dt.bfloat16

_CACHE = {}


def _build():
    nc = bacc.Bacc('TRN2', debug=False, num_devices=NCORES)
    xT = nc.dram_tensor('xT', [H, S], BF16, kind='ExternalInput')
    yT = nc.dram_tensor('yT', [H, S], BF16, kind='ExternalInput')
    wq = nc.dram_tensor('wq', [H, C], BF16, kind='ExternalInput')
    wk = nc.dram_tensor('wk', [H, C], BF16, kind='ExternalInput')
    wv = nc.dram_tensor('wv', [H, C], BF16, kind='ExternalInput')
    wo = nc.dram_tensor('wo', [C, H], BF16, kind='ExternalInput')
    ebias = nc.dram_tensor('ebias', [128, SK], F32, kind='ExternalInput')
    out = nc.dram_tensor('out', [S, H], F32, kind='ExternalOutput')

    with tile.TileContext(nc) as tc, ExitStack() as ctx:
        res = ctx.enter_context(tc.tile_pool(name='res', bufs=1))
        expool = ctx.enter_context(tc.tile_pool(name='expool', bufs=4))
        ctxpool = ctx.enter_context(tc.tile_pool(name='ctxpool', bufs=2))
        small = ctx.enter_context(tc.tile_pool(name='small', bufs=3))
        outpool = ctx.enter_context(tc.tile_pool(name='outpool', bufs=3))
        ps_qk = ctx.enter_context(tc.tile_pool(name='ps_qk', bufs=2, space='PSUM'))
        ps_pv = ctx.enter_context(tc.tile_pool(name='ps_pv', bufs=4, space='PSUM'))

        # ---- exp table preload: dummy activation at t~0 ----
        warm_in = small.tile([128, 8], F32, tag='warm')
        nc.vector.memset(warm_in, 0.0)
        warm_out = small.tile([128, 8], BF16, tag='warm2')
        nc.scalar.activation(warm_out, warm_in, mybir.ActivationFunctionType.Exp)

        # ---- input DMAs, ordered for earliest prefix start ----
        eb = res.tile([128, SK], F32, tag='eb')
        nc.sync.dma_start(out=eb, in_=ebias.ap())
        wk_r = res.tile([128, KT_H, C], BF16, tag='wk')
        wq_r = res.tile([128, KT_H, C], BF16, tag='wq')
        wv_r = res.tile([128, KT_H, C], BF16, tag='wv')
        y_r = res.tile([128, KT_H, S], BF16, tag='y_r')
        x_r = res.tile([128, KT_H, S], BF16, tag='x_r')

        def wslice(t, c0, c1):
            return t.ap()[:, c0:c1].rearrange('(t p) c -> p t c', p=128)

        nc.sync.dma_start(out=wk_r[:, :, 0:128], in_=wslice(wk, 0, 128))
        nc.sync.dma_start(out=wv_r, in_=wv.ap().rearrange('(t p) c -> p t c', p=128))
        nc.sync.dma_start(out=wq_r[:, :, 0:128], in_=wslice(wq, 0, 128))
        nc.sync.dma_start(out=wk_r[:, :, 128:256], in_=wslice(wk, 128, 256))
        nc.sync.dma_start(out=wq_r[:, :, 128:256], in_=wslice(wq, 128, 256))
        for q in range(4):
            qs = slice(q * JBLK, (q + 1) * JBLK)
            nc.scalar.dma_start(
                out=y_r[:, :, qs],
                in_=yT.ap()[:, qs].rearrange('(t p) s -> p t s', p=128))
            nc.scalar.dma_start(
                out=x_r[:, :, qs],
                in_=xT.ap()[:, qs].rearrange('(t p) s -> p t s', p=128))
        wo_r = res.tile([128, 2, H], BF16, tag='wo')
        nc.sync.dma_start(out=wo_r, in_=wo.ap().rearrange('(t p) n -> p t n', p=128))

        # ---- resident activations ----
        QT = [res.tile([128, S], BF16, tag=f'qt{p}', name=f'qt{p}') for p in range(2)]
        KTs = [res.tile([128, S], BF16, tag=f'kt{p}', name=f'kt{p}') for p in range(2)]
        v_sb = [res.tile([128, HPC, DH + 1], BF16, tag=f'v{i}', name=f'v{i}')
                for i in range(SK)]
        ones4 = res.tile([128, HPC, 1], F32, tag='ones4')
        nc.vector.memset(ones4, 1.0)

        # ---- projection groups (8 matmuls + eviction), run direct or woven ----
        gid = [0]

        def qk_group(which, p, j4):
            w_r = wq_r if which == 'q' else wk_r
            src = x_r if which == 'q' else y_r
            dest = QT[p] if which == 'q' else KTs[p]
            js = slice(j4 * JBLK, (j4 + 1) * JBLK)
            cs = slice(p * 128, (p + 1) * 128)
            gid[0] += 1
            ps = ps_pv.tile([128, JBLK], F32, tag='pv', name=f'g{gid[0]}')
            items = []
            for k in range(KT_H):
                def mm(k=k):
                    nc.tensor.matmul(ps, w_r[:, k, cs], src[:, k, js],
                                     start=(k == 0), stop=(k == KT_H - 1))
                items.append(mm)

            def fin():
                nc.vector.tensor_copy(dest[:, js], ps)
            items.append(fin)
            return items

        def v_group(j4, m):
            sub = j4 * 4 + m
            s0 = j4 * JBLK + m * 128
            gid[0] += 1
            ps = ps_pv.tile([128, JBLK], F32, tag='pv', name=f'g{gid[0]}')
            items = []
            for k in range(KT_H):
                def mm(k=k):
                    nc.tensor.matmul(ps[:, 0:C], y_r[:, k, s0:s0 + 128],
                                     wv_r[:, k, :],
                                     start=(k == 0), stop=(k == KT_H - 1))
                items.append(mm)

            def fin():
                nc.vector.tensor_scalar_mul(
                    v_sb[sub][:, :, 0:DH],
                    ps[:, 0:C].rearrange('p (h c) -> p h c', h=HPC),
                    eb[:, sub:sub + 1])
                nc.gpsimd.tensor_scalar_mul(v_sb[sub][:, :, DH:DH + 1], ones4,
                                            eb[:, sub:sub + 1])
            items.append(fin)
            return items

        # prefix: only what attention steps (pidx=0, kk=0..3) touch
        for grp in ([qk_group('k', 0, 0)] + [qk_group('q', 0, 0)]
                    + [v_group(0, m) for m in range(4)]):
            for it in grp:
                it()

        # woven into the attention k-loop under consumption deadlines:
        # dl = the step whose emission (QK lookahead / PV) first needs the
        # group's output.  Service runs at the top of each step, emitting
        # every overdue item plus a small budget of future ones, so writers
        # are always emitted before their readers (Tile's semaphore
        # assignment relies on program order matching dataflow order).
        weave = []
        _seq = [0]
        def _push(dl, groups):
            for grp in groups:
                for it in grp:
                    _seq[0] += 1
                    bisect.insort(weave, (dl, _seq[0], it))
        _push(2, [qk_group('k', 0, 1)])
        _push(4, [v_group(1, 0)]); _push(5, [v_group(1, 1)])
        _push(6, [v_group(1, 2), qk_group('k', 0, 2)])
        _push(7, [v_group(1, 3)])
        _push(8, [v_group(2, 0)]); _push(9, [v_group(2, 1)])
        _push(10, [qk_group('k', 0, 3), v_group(2, 2)])
        _push(11, [v_group(2, 3)])
        _push(12, [v_group(3, 0)]); _push(13, [v_group(3, 1)])
        _push(14, [qk_group('k', 1, 0), qk_group('q', 1, 0), v_group(3, 2)])
        _push(15, [v_group(3, 3)])
        _push(18, [qk_group('k', 1, 1)])
        _push(22, [qk_group('k', 1, 2)])
        _push(26, [qk_group('k', 1, 3)])
        _push(30, [qk_group('q', 0, 1)])
        _push(46, [qk_group('q', 1, 1)])
        _push(62, [qk_group('q', 0, 2)])
        _push(78, [qk_group('q', 1, 2)])
        _push(94, [qk_group('q', 0, 3)])
        _push(110, [qk_group('q', 1, 3)])

        def weave_service(step, budget=2):
            n = 0
            while weave and (weave[0][0] <= step or n < budget):
                _, _, it = weave.pop(0)
                it()
                n += 1

        # ---- attention + output projection ----
        def out_groups(J, ctx_tiles, queues):
            groups = []
            for i, (m, n) in enumerate([(m, n) for m in range(4) for n in range(2)]):
                def grp(m=m, n=n, i=i):
                    ms = slice(m * 128, (m + 1) * 128)
                    ns = slice(n * JBLK, (n + 1) * JBLK)
                    pso = ps_pv.tile([128, JBLK], F32, tag='pv', name=f'o{J}_{m}_{n}')
                    for p in range(2):
                        nc.tensor.matmul(pso, ctx_tiles[p][:, ms], wo_r[:, p, ns],
                                         start=(p == 0), stop=(p == 1))
                    ob = outpool.tile([128, JBLK], F32, tag='ob')
                    nc.vector.tensor_copy(ob, pso)
                    eng = queues[i % len(queues)]
                    eng.dma_start(out=out.ap()[J * JBLK + m * 128:
                                               J * JBLK + (m + 1) * 128, ns],
                                  in_=ob)
                groups.append(grp)
            return groups

        pairs = [(J, p) for J in range(NJ) for p in range(2)]
        psl_q = deque()

        def emit_qk(pidx, kk):
            if pidx >= len(pairs):
                return
            J, p = pairs[pidx]
            js = slice(J * JBLK, (J + 1) * JBLK)
            kks = slice(kk * 128, (kk + 1) * 128)
            psl = ps_qk.tile([128, 2 * JBLK], F32, tag='qk',
                             name=f'psl{pidx}_{kk}')
            nc.tensor.matmul(psl[:, 0:JBLK],
                             KTs[p][0:64, kks], QT[p][0:64, js],
                             start=True, stop=True, tile_position=(0, 0))
            nc.tensor.matmul(psl[:, JBLK:2 * JBLK],
                             KTs[p][64:128, kks], QT[p][64:128, js],
                             start=True, stop=True, tile_position=(64, 0))
            psl_q.append(psl)

        emit_qk(0, 0)
        emit_qk(0, 1)
        for J in range(NJ):
            ctx_tiles = []
            for p in range(2):
                pidx = J * 2 + p
                pv0 = ps_pv.tile([128, JBLK], F32, tag='pv')
                pv1 = ps_pv.tile([128, JBLK], F32, tag='pv')
                for kk in range(SK):
                    step = pidx * SK + kk
                    # service the weave FIRST so every producer is emitted
                    # before the QK lookahead / PV that consumes it
                    weave_service(step)
                    # QK two steps ahead, crossing pair boundaries so the
                    # next pair's logits are queued before this pair's tail
                    if kk + 2 < SK:
                        emit_qk(pidx, kk + 2)
                    else:
                        emit_qk(pidx + 1, kk + 2 - SK)
                    psl = psl_q.popleft()
                    ex = expool.tile([128, 2 * JBLK], BF16, tag='ex')
                    nc.scalar.activation(ex, psl, mybir.ActivationFunctionType.Exp)
                    for hh, pv in enumerate((pv0, pv1)):
                        hcol = 2 * p + hh
                        nc.tensor.matmul(
                            pv[0:DH + 1, :],
                            v_sb[kk][:, hcol, :],
                            ex[:, hh * JBLK:(hh + 1) * JBLK],
                            start=(kk == 0), stop=(kk == SK - 1))
                # normalize: ctxT[d, q] * (1/denom[q]) via partition broadcast
                ct = ctxpool.tile([128, JBLK], BF16, tag=f'ctx{p}')
                stage = []
                for hh, pv in enumerate((pv0, pv1)):
                    rawct = small.tile([128, JBLK], F32, tag='rawct')
                    nc.vector.tensor_copy(rawct[0:DH + 1, :], pv[0:DH + 1, :])
                    rec = small.tile([128, JBLK], F32, tag='rec')
                    nc.vector.reciprocal_approx_fast(rec[0:DH + 1, :],
                                                     rawct[0:DH + 1, :])
                    bcs = small.tile([128, JBLK], F32, tag='bcs')
                    nc.sync.dma_start(out=bcs[0:1, :], in_=rec[DH:DH + 1, :])
                    bc = small.tile([128, JBLK], F32, tag='bc')
                    nc.gpsimd.partition_broadcast(bc[0:DH, :], bcs[0:1, :])
                    stage.append((rawct, bc))
                for hh, (rawct, bc) in enumerate(stage):
                    if hh == 0:
                        nc.vector.tensor_mul(ct[0:DH, :], rawct[0:DH, :], bc[0:DH, :])
                    else:
                        tmp = small.tile([128, JBLK], BF16, tag='tmp')
                        nc.vector.tensor_mul(tmp[0:DH, :], rawct[0:DH, :], bc[0:DH, :])
                        nc.sync.dma_start(out=ct[DH:128, :], in_=tmp[0:DH, :])
                ctx_tiles.append(ct)
            if J < NJ - 1:
                # out-proj woven into later steps; must fully emit before
                # ct(J+2) overwrites this J's ctx tiles (~step 32*J+79)
                _push(min(32 * J + 64, 120),
                      [[g] for g in out_groups(J, ctx_tiles, [nc.sync])])
            else:
                weave_service(10 ** 9, budget=len(weave))
                for g in out_groups(J, ctx_tiles, [nc.sync, nc.scalar]):
                    g()

    nc.compile()
    return nc


def _get_nc():
    if 'nc' not in _CACHE:
        _CACHE['nc'] = _build()
    return _CACHE['nc']


def shard_inputs(x, y, bias, wq, wk, wv, wo):
    """Build the 8 per-core input maps from full inputs."""
    scale = (H // NH) ** -0.5
    wqs = (wq * scale).astype(np.float32)
    bf = ml_dtypes.bfloat16
    in_maps = []
    for c in range(NCORES):
        b = c // (NCORES // B)
        g = c % (NCORES // B)
        cols = slice(g * C, (g + 1) * C)
        eb = np.exp(bias[b, 0, 0, :].astype(np.float64)).astype(np.float32)
        in_maps.append({
            'xT': np.ascontiguousarray(x[b].T.astype(bf)),
            'yT': np.ascontiguousarray(y[b].T.astype(bf)),
            'wq': np.ascontiguousarray(wqs[:, cols].astype(bf)),
            'wk': np.ascontiguousarray(wk[:, cols].astype(bf)),
            'wv': np.ascontiguousarray(wv[:, cols].astype(bf)),
            'wo': np.ascontiguousarray(wo[cols, :].astype(bf)),
            'ebias': np.ascontiguousarray(eb.reshape(SK, 128).T),
        })
    return in_maps


def kernel(x, y, bias, wq, wk, wv, wo, _trace=False):
    x, y, bias = np.asarray(x), np.asarray(y), np.asarray(bias)
    wq, wk, wv, wo = (np.asarray(t) for t in (wq, wk, wv, wo))
    nc = _get_nc()
    in_maps = shard_inputs(x, y, bias, wq, wk, wv, wo)
    kw = {}
    if _trace:
        kw = dict(trace=True, stitch_traces=False)
    res = bass_utils.run_bass_kernel_spmd(nc, in_maps, core_ids=list(range(NCORES)), **kw)
    full = np.zeros((B, S, H), dtype=np.float64)
    for c in range(NCORES):
        full[c // (NCORES // B)] += res.results[c]['out'].astype(np.float64)
    if _trace:
        _CACHE['last_results'] = res
    return full.astype(np.float32)


# revision 30
# speedup vs baseline: 1.1109x; 1.1109x over previous
"""Multi-head attention (B=2, S=2048, H=1024, 16 heads) on 8 TRN2 NeuronCores.

Sharding: data parallel on batch (2) x tensor parallel on heads (4 heads/core,
Megatron column-split qkv, row-split wo). Host pre-transposes x/y, pre-scales
wq by dh^-0.5, and sum-reduces the 4 partial outputs per batch element.

Per-core kernel (v2): the ScalarE exp stream (1 elem/cycle/lane) paces the
steady state, so the wins are at the edges:
  - startup: a dummy exp preloads the ACT table at t~0; inputs arrive as
    split DMAs on two queues ordered so the minimal prefix (K[j4=0], Q[J=0],
    V[j4=0]) starts as soon as its bytes land; the remaining projection
    groups are woven into the early attention steps (ordered deque, budgeted
    emission; order respects each group's consumption deadline).
  - steady state: row-tiled (2-head packed) QK^T -> psl psum [128,1024] ->
    one exp per key block -> PV matmul with fused ones*exp(bias) denominator
    column (M=65) -> reciprocal + gpsimd partition-broadcast normalize ->
    pair-stacked bf16 ctx -> deferred output projection woven into later
    steps.
  - tail: the final q-block's output projection is emitted directly with
    its DMAs spread across queues.
"""
import sys
sys.path.insert(0, '/opt/trn_rl_repo')
import bisect
from collections import deque
from contextlib import ExitStack

import numpy as np
import ml_dtypes

import concourse.bacc as bacc
import concourse.tile as tile
from concourse import mybir
from concourse import bass_utils

B, S, H, NH = 2, 2048, 1024, 16
DH = H // NH            # 64
NCORES = 8
HPC = NH // (NCORES // B)   # 4 heads per core
C = HPC * DH            # 256 projected cols per core
KT_H = H // 128         # 8 contraction tiles over H
SK = S // 128           # 16 s-subtiles
JBLK = 512
NJ = S // JBLK          # 4 q-blocks
F32 = mybir.dt.float32
BF16 = mybir.dt.bfloat16

_CACHE = {}


def _build():
    nc = bacc.Bacc('TRN2', debug=False, num_devices=NCORES)
    xT = nc.dram_tensor('xT', [H, S], BF16, kind='ExternalInput')
    yT = nc.dram_tensor('yT', [H, S], BF16, kind='ExternalInput')
    wq = nc.dram_tensor('wq', [H, C], BF16, kind='ExternalInput')
    wk = nc.dram_tensor('wk', [H, C], BF16, kind='ExternalInput')
    wv = nc.dram_tensor('wv', [H, C], BF16, kind='ExternalInput')
    wo = nc.dram_tensor('wo', [C, H], BF16, kind='ExternalInput')
    ebias = nc.dram_tensor('ebias', [128, SK], F32, kind='ExternalInput')
    out = nc.dram_tensor('out', [S, H], F32, kind='ExternalOutput')

    with tile.TileContext(nc) as tc, ExitStack() as ctx:
        res = ctx.enter_context(tc.tile_pool(name='res', bufs=1))
        expool = ctx.enter_context(tc.tile_pool(name='expool', bufs=4))
        ctxpool = ctx.enter_context(tc.tile_pool(name='ctxpool', bufs=2))
        small = ctx.enter_context(tc.tile_pool(name='small', bufs=3))
        outpool = ctx.enter_context(tc.tile_pool(name='outpool', bufs=3))
        ps_qk = ctx.enter_context(tc.tile_pool(name='ps_qk', bufs=2, space='PSUM'))
        ps_pv = ctx.enter_context(tc.tile_pool(name='ps_pv', bufs=4, space='PSUM'))

        # ---- exp table preload: dummy activation at t~0 ----
        warm_in = small.tile([128, 8], F32, tag='warm')
        nc.vector.memset(warm_in, 0.0)
        warm_out = small.tile([128, 8], BF16, tag='warm2')
        nc.scalar.activation(warm_out, warm_in, mybir.ActivationFunctionType.Exp)

        # ---- input DMAs, ordered for earliest prefix start ----
        eb = res.tile([128, SK], F32, tag='eb')
        nc.sync.dma_start(out=eb, in_=ebias.ap())
        wk_r = res.tile([128, KT_H, C], BF16, tag='wk')
        wq_r = res.tile([128, KT_H, C], BF16, tag='wq')
        wv_r = res.tile([128, KT_H, C], BF16, tag='wv')

        def wslice(t, c0, c1):
            return t.ap()[:, c0:c1].rearrange('(t p) c -> p t c', p=128)

        nc.sync.dma_start(out=wk_r[:, :, 0:128], in_=wslice(wk, 0, 128))
        nc.sync.dma_start(out=wv_r, in_=wv.ap().rearrange('(t p) c -> p t c', p=128))
        nc.sync.dma_start(out=wq_r[:, :, 0:128], in_=wslice(wq, 0, 128))
        nc.sync.dma_start(out=wk_r[:, :, 128:256], in_=wslice(wk, 128, 256))
        nc.sync.dma_start(out=wq_r[:, :, 128:256], in_=wslice(wq, 128, 256))
        xT_ap, yT_ap = xT.ap(), yT.ap()
        HB = S // 2
        xts = [[res.tile([128, HB], BF16, tag=f'xts{k}_{j}', name=f'xts{k}_{j}')
                for j in range(2)] for k in range(KT_H)]
        yts = [[res.tile([128, HB], BF16, tag=f'yts{k}_{j}', name=f'yts{k}_{j}')
                for j in range(2)] for k in range(KT_H)]
        for j in range(2):
            hs = slice(j * HB, (j + 1) * HB)
            for k in range(KT_H):
                nc.scalar.dma_start(out=yts[k][j],
                                    in_=yT_ap[k * 128:(k + 1) * 128, hs])
            for k in range(KT_H):
                nc.scalar.dma_start(out=xts[k][j],
                                    in_=xT_ap[k * 128:(k + 1) * 128, hs])
        wo_r = res.tile([128, 2, H], BF16, tag='wo')
        nc.sync.dma_start(out=wo_r, in_=wo.ap().rearrange('(t p) n -> p t n', p=128))

        # ---- resident activations ----
        QT = [res.tile([128, S], BF16, tag=f'qt{p}', name=f'qt{p}') for p in range(2)]
        KTs = [res.tile([128, S], BF16, tag=f'kt{p}', name=f'kt{p}') for p in range(2)]
        v_sb = [res.tile([128, HPC, DH + 1], BF16, tag=f'v{i}', name=f'v{i}')
                for i in range(SK)]
        ones4 = res.tile([128, HPC, 1], F32, tag='ones4')
        nc.vector.memset(ones4, 1.0)

        # ---- projection groups (8 matmuls + eviction), run direct or woven ----
        gid = [0]

        def qk_group(which, p, j4):
            w_r = wq_r if which == 'q' else wk_r
            src = xts if which == 'q' else yts
            dest = QT[p] if which == 'q' else KTs[p]
            js = slice(j4 * JBLK, (j4 + 1) * JBLK)
            hj = slice((j4 % 2) * JBLK, (j4 % 2 + 1) * JBLK)
            cs = slice(p * 128, (p + 1) * 128)
            gid[0] += 1
            ps = ps_pv.tile([128, JBLK], F32, tag='pv', name=f'g{gid[0]}')
            items = []
            for k in range(KT_H):
                def mm(k=k):
                    nc.tensor.matmul(ps, w_r[:, k, cs], src[k][j4 // 2][:, hj],
                                     start=(k == 0), stop=(k == KT_H - 1))
                items.append(mm)

            def fin():
                nc.vector.tensor_copy(dest[:, js], ps)
            items.append(fin)
            return items

        def v_group(j4, m):
            sub = j4 * 4 + m
            hj0 = (j4 % 2) * JBLK + m * 128
            gid[0] += 1
            ps = ps_pv.tile([128, JBLK], F32, tag='pv', name=f'g{gid[0]}')
            items = []
            for k in range(KT_H):
                def mm(k=k):
                    nc.tensor.matmul(ps[:, 0:C],
                                     yts[k][j4 // 2][:, hj0:hj0 + 128],
                                     wv_r[:, k, :],
                                     start=(k == 0), stop=(k == KT_H - 1))
                items.append(mm)

            def fin():
                nc.vector.tensor_scalar_mul(
                    v_sb[sub][:, :, 0:DH],
                    ps[:, 0:C].rearrange('p (h c) -> p h c', h=HPC),
                    eb[:, sub:sub + 1])
                nc.gpsimd.tensor_scalar_mul(v_sb[sub][:, :, DH:DH + 1], ones4,
                                            eb[:, sub:sub + 1])
            items.append(fin)
            return items

        # prefix: only what attention steps (pidx=0, kk=0..3) touch
        for grp in ([qk_group('k', 0, 0)] + [qk_group('q', 0, 0)]
                    + [v_group(0, m) for m in range(4)]):
            for it in grp:
                it()

        # woven into the attention k-loop under consumption deadlines:
        # dl = the step whose emission (QK lookahead / PV) first needs the
        # group's output.  Service runs at the top of each step, emitting
        # every overdue item plus a small budget of future ones, so writers
        # are always emitted before their readers (Tile's semaphore
        # assignment relies on program order matching dataflow order).
        weave = []
        _seq = [0]
        def _push(dl, groups):
            for grp in groups:
                for it in grp:
                    _seq[0] += 1
                    bisect.insort(weave, (dl, _seq[0], it))
        _push(2, [qk_group('k', 0, 1)])
        _push(4, [v_group(1, 0)]); _push(5, [v_group(1, 1)])
        _push(6, [v_group(1, 2), qk_group('k', 0, 2)])
        _push(7, [v_group(1, 3)])
        _push(8, [v_group(2, 0)]); _push(9, [v_group(2, 1)])
        _push(10, [qk_group('k', 0, 3), v_group(2, 2)])
        _push(11, [v_group(2, 3)])
        _push(12, [v_group(3, 0)]); _push(13, [v_group(3, 1)])
        _push(14, [qk_group('k', 1, 0), qk_group('q', 1, 0), v_group(3, 2)])
        _push(15, [v_group(3, 3)])
        _push(18, [qk_group('k', 1, 1)])
        _push(22, [qk_group('k', 1, 2)])
        _push(26, [qk_group('k', 1, 3)])
        _push(30, [qk_group('q', 0, 1)])
        _push(46, [qk_group('q', 1, 1)])
        _push(62, [qk_group('q', 0, 2)])
        _push(78, [qk_group('q', 1, 2)])
        _push(94, [qk_group('q', 0, 3)])
        _push(110, [qk_group('q', 1, 3)])

        def weave_service(step, budget=2):
            n = 0
            while weave and (weave[0][0] <= step or n < budget):
                _, _, it = weave.pop(0)
                it()
                n += 1

        # ---- attention + output projection ----
        def out_groups(J, ctx_tiles, queues):
            groups = []
            for i, (m, n) in enumerate([(m, n) for m in range(4) for n in range(2)]):
                def grp(m=m, n=n, i=i):
                    ms = slice(m * 128, (m + 1) * 128)
                    ns = slice(n * JBLK, (n + 1) * JBLK)
                    pso = ps_pv.tile([128, JBLK], F32, tag='pv', name=f'o{J}_{m}_{n}')
                    for p in range(2):
                        nc.tensor.matmul(pso, ctx_tiles[p][:, ms], wo_r[:, p, ns],
                                         start=(p == 0), stop=(p == 1))
                    ob = outpool.tile([128, JBLK], F32, tag='ob')
                    nc.vector.tensor_copy(ob, pso)
                    eng = queues[i % len(queues)]
                    eng.dma_start(out=out.ap()[J * JBLK + m * 128:
                                               J * JBLK + (m + 1) * 128, ns],
                                  in_=ob)
                groups.append(grp)
            return groups

        pairs = [(J, p) for J in range(NJ) for p in range(2)]
        psl_q = deque()

        def emit_qk(pidx, kk):
            if pidx >= len(pairs):
                return
            J, p = pairs[pidx]
            js = slice(J * JBLK, (J + 1) * JBLK)
            kks = slice(kk * 128, (kk + 1) * 128)
            psl = ps_qk.tile([128, 2 * JBLK], F32, tag='qk',
                             name=f'psl{pidx}_{kk}')
            nc.tensor.matmul(psl[:, 0:JBLK],
                             KTs[p][0:64, kks], QT[p][0:64, js],
                             start=True, stop=True, tile_position=(0, 0))
            nc.tensor.matmul(psl[:, JBLK:2 * JBLK],
                             KTs[p][64:128, kks], QT[p][64:128, js],
                             start=True, stop=True, tile_position=(64, 0))
            psl_q.append(psl)

        emit_qk(0, 0)
        emit_qk(0, 1)
        for J in range(NJ):
            ctx_tiles = []
            for p in range(2):
                pidx = J * 2 + p
                pv0 = ps_pv.tile([128, JBLK], F32, tag='pv')
                pv1 = ps_pv.tile([128, JBLK], F32, tag='pv')
                for kk in range(SK):
                    step = pidx * SK + kk
                    # service the weave FIRST so every producer is emitted
                    # before the QK lookahead / PV that consumes it
                    weave_service(step)
                    # QK two steps ahead, crossing pair boundaries so the
                    # next pair's logits are queued before this pair's tail
                    if kk + 2 < SK:
                        emit_qk(pidx, kk + 2)
                    else:
                        emit_qk(pidx + 1, kk + 2 - SK)
                    psl = psl_q.popleft()
                    ex = expool.tile([128, 2 * JBLK], BF16, tag='ex')
                    nc.scalar.activation(ex, psl, mybir.ActivationFunctionType.Exp)
                    for hh, pv in enumerate((pv0, pv1)):
                        hcol = 2 * p + hh
                        nc.tensor.matmul(
                            pv[0:DH + 1, :],
                            v_sb[kk][:, hcol, :],
                            ex[:, hh * JBLK:(hh + 1) * JBLK],
                            start=(kk == 0), stop=(kk == SK - 1))
                # normalize: ctxT[d, q] * (1/denom[q]) via partition broadcast
                ct = ctxpool.tile([128, JBLK], BF16, tag=f'ctx{p}')
                stage = []
                for hh, pv in enumerate((pv0, pv1)):
                    rawct = small.tile([128, JBLK], F32, tag='rawct')
                    nc.vector.tensor_copy(rawct[0:DH + 1, :], pv[0:DH + 1, :])
                    rec = small.tile([128, JBLK], F32, tag='rec')
                    nc.vector.reciprocal_approx_fast(rec[0:DH + 1, :],
                                                     rawct[0:DH + 1, :])
                    bcs = small.tile([128, JBLK], F32, tag='bcs')
                    nc.sync.dma_start(out=bcs[0:1, :], in_=rec[DH:DH + 1, :])
                    bc = small.tile([128, JBLK], F32, tag='bc')
                    nc.gpsimd.partition_broadcast(bc[0:DH, :], bcs[0:1, :])
                    stage.append((rawct, bc))
                for hh, (rawct, bc) in enumerate(stage):
                    if hh == 0:
                        nc.vector.tensor_mul(ct[0:DH, :], rawct[0:DH, :], bc[0:DH, :])
                    else:
                        tmp = small.tile([128, JBLK], BF16, tag='tmp')
                        nc.vector.tensor_mul(tmp[0:DH, :], rawct[0:DH, :], bc[0:DH, :])
                        nc.sync.dma_start(out=ct[DH:128, :], in_=tmp[0:DH, :])
                ctx_tiles.append(ct)
            if J < NJ - 1:
                # out-proj woven into later steps; must fully emit before
                # ct(J+2) overwrites this J's ctx tiles (~step 32*J+79)
                _push(min(32 * J + 64, 120),
                      [[g] for g in out_groups(J, ctx_tiles, [nc.sync])])
            else:
                weave_service(10 ** 9, budget=len(weave))
                for g in out_groups(J, ctx_tiles, [nc.sync, nc.scalar]):
                    g()

    nc.compile()
    return nc


def _get_nc():
    if 'nc' not in _CACHE:
        _CACHE['nc'] = _build()
    return _CACHE['nc']


def shard_inputs(x, y, bias, wq, wk, wv, wo):
    """Build the 8 per-core input maps from full inputs."""
    scale = (H // NH) ** -0.5
    wqs = (wq * scale).astype(np.float32)
    bf = ml_dtypes.bfloat16
    in_maps = []
    for c in range(NCORES):
        b = c // (NCORES // B)
        g = c % (NCORES // B)
        cols = slice(g * C, (g + 1) * C)
        eb = np.exp(bias[b, 0, 0, :].astype(np.float64)).astype(np.float32)
        in_maps.append({
            'xT': np.ascontiguousarray(x[b].T.astype(bf)),
            'yT': np.ascontiguousarray(y[b].T.astype(bf)),
            'wq': np.ascontiguousarray(wqs[:, cols].astype(bf)),
            'wk': np.ascontiguousarray(wk[:, cols].astype(bf)),
            'wv': np.ascontiguousarray(wv[:, cols].astype(bf)),
            'wo': np.ascontiguousarray(wo[cols, :].astype(bf)),
            'ebias': np.ascontiguousarray(eb.reshape(SK, 128).T),
        })
    return in_maps


def kernel(x, y, bias, wq, wk, wv, wo, _trace=False):
    x, y, bias = np.asarray(x), np.asarray(y), np.asarray(bias)
    wq, wk, wv, wo = (np.asarray(t) for t in (wq, wk, wv, wo))
    nc = _get_nc()
    in_maps = shard_inputs(x, y, bias, wq, wk, wv, wo)
    kw = {}
    if _trace:
        kw = dict(trace=True, stitch_traces=False)
    res = bass_utils.run_bass_kernel_spmd(nc, in_maps, core_ids=list(range(NCORES)), **kw)
    full = np.zeros((B, S, H), dtype=np.float64)
    for c in range(NCORES):
        full[c // (NCORES // B)] += res.results[c]['out'].astype(np.float64)
    if _trace:
        _CACHE['last_results'] = res
    return full.astype(np.float32)


# revision 31
# speedup vs baseline: 1.1241x; 1.0119x over previous
"""Multi-head attention (B=2, S=2048, H=1024, 16 heads) on 8 TRN2 NeuronCores.

Sharding: data parallel on batch (2) x tensor parallel on heads (4 heads/core,
Megatron column-split qkv, row-split wo). Host pre-transposes x/y, pre-scales
wq by dh^-0.5, and sum-reduces the 4 partial outputs per batch element.

Per-core kernel (v2): the ScalarE exp stream (1 elem/cycle/lane) paces the
steady state, so the wins are at the edges:
  - startup: a dummy exp preloads the ACT table at t~0; inputs arrive as
    split DMAs on two queues ordered so the minimal prefix (K[j4=0], Q[J=0],
    V[j4=0]) starts as soon as its bytes land; the remaining projection
    groups are woven into the early attention steps (ordered deque, budgeted
    emission; order respects each group's consumption deadline).
  - steady state: row-tiled (2-head packed) QK^T -> psl psum [128,1024] ->
    one exp per key block -> PV matmul with fused ones*exp(bias) denominator
    column (M=65) -> reciprocal + gpsimd partition-broadcast normalize ->
    pair-stacked bf16 ctx -> deferred output projection woven into later
    steps.
  - tail: the final q-block's output projection is emitted directly with
    its DMAs spread across queues.
"""
import sys
sys.path.insert(0, '/opt/trn_rl_repo')
import bisect
from collections import deque
from contextlib import ExitStack

import numpy as np
import ml_dtypes

import concourse.bacc as bacc
import concourse.tile as tile
from concourse import mybir
from concourse import bass_utils

B, S, H, NH = 2, 2048, 1024, 16
DH = H // NH            # 64
NCORES = 8
HPC = NH // (NCORES // B)   # 4 heads per core
C = HPC * DH            # 256 projected cols per core
KT_H = H // 128         # 8 contraction tiles over H
SK = S // 128           # 16 s-subtiles
JBLK = 512
NJ = S // JBLK          # 4 q-blocks
F32 = mybir.dt.float32
BF16 = mybir.dt.bfloat16

_CACHE = {}


def _build():
    nc = bacc.Bacc('TRN2', debug=False, num_devices=NCORES)
    xT = nc.dram_tensor('xT', [H, S], BF16, kind='ExternalInput')
    yT = nc.dram_tensor('yT', [H, S], BF16, kind='ExternalInput')
    wq = nc.dram_tensor('wq', [H, C], BF16, kind='ExternalInput')
    wk = nc.dram_tensor('wk', [H, C], BF16, kind='ExternalInput')
    wv = nc.dram_tensor('wv', [H, C], BF16, kind='ExternalInput')
    wo = nc.dram_tensor('wo', [C, H], BF16, kind='ExternalInput')
    ebias = nc.dram_tensor('ebias', [128, SK], F32, kind='ExternalInput')
    out = nc.dram_tensor('out', [S, H], F32, kind='ExternalOutput')

    with tile.TileContext(nc) as tc, ExitStack() as ctx:
        res = ctx.enter_context(tc.tile_pool(name='res', bufs=1))
        expool = ctx.enter_context(tc.tile_pool(name='expool', bufs=4))
        ctxpool = ctx.enter_context(tc.tile_pool(name='ctxpool', bufs=2))
        small = ctx.enter_context(tc.tile_pool(name='small', bufs=3))
        outpool = ctx.enter_context(tc.tile_pool(name='outpool', bufs=3))
        ps_qk = ctx.enter_context(tc.tile_pool(name='ps_qk', bufs=2, space='PSUM'))
        ps_pv = ctx.enter_context(tc.tile_pool(name='ps_pv', bufs=4, space='PSUM'))

        # ---- exp table preload: dummy activation at t~0 ----
        warm_in = small.tile([128, 8], F32, tag='warm')
        nc.vector.memset(warm_in, 0.0)
        warm_out = small.tile([128, 8], BF16, tag='warm2')
        nc.scalar.activation(warm_out, warm_in, mybir.ActivationFunctionType.Exp)

        # ---- input DMAs, ordered for earliest prefix start ----
        eb = res.tile([128, SK], F32, tag='eb')
        nc.sync.dma_start(out=eb, in_=ebias.ap())
        wk_r = res.tile([128, KT_H, C], BF16, tag='wk')
        wq_r = res.tile([128, KT_H, C], BF16, tag='wq')
        wv_r = res.tile([128, KT_H, C], BF16, tag='wv')

        def wslice(t, c0, c1):
            return t.ap()[:, c0:c1].rearrange('(t p) c -> p t c', p=128)

        nc.sync.dma_start(out=wk_r[:, :, 0:128], in_=wslice(wk, 0, 128))
        nc.sync.dma_start(out=wv_r, in_=wv.ap().rearrange('(t p) c -> p t c', p=128))
        nc.sync.dma_start(out=wq_r[:, :, 0:128], in_=wslice(wq, 0, 128))
        nc.sync.dma_start(out=wk_r[:, :, 128:256], in_=wslice(wk, 128, 256))
        nc.sync.dma_start(out=wq_r[:, :, 128:256], in_=wslice(wq, 128, 256))
        xT_ap, yT_ap = xT.ap(), yT.ap()
        HB = S // 2
        xts = [[res.tile([128, HB], BF16, tag=f'xts{k}_{j}', name=f'xts{k}_{j}')
                for j in range(2)] for k in range(KT_H)]
        yts = [[res.tile([128, HB], BF16, tag=f'yts{k}_{j}', name=f'yts{k}_{j}')
                for j in range(2)] for k in range(KT_H)]
        # y on the scalar queue, x on sync: the two halves transfer in
        # parallel so the K and Q prefix groups unblock together
        for j in range(2):
            hs = slice(j * HB, (j + 1) * HB)
            for k in range(KT_H):
                nc.scalar.dma_start(out=yts[k][j],
                                    in_=yT_ap[k * 128:(k + 1) * 128, hs])
                nc.sync.dma_start(out=xts[k][j],
                                  in_=xT_ap[k * 128:(k + 1) * 128, hs])
        wo_r = res.tile([128, 2, H], BF16, tag='wo')
        nc.sync.dma_start(out=wo_r, in_=wo.ap().rearrange('(t p) n -> p t n', p=128))

        # ---- resident activations ----
        QT = [res.tile([128, S], BF16, tag=f'qt{p}', name=f'qt{p}') for p in range(2)]
        KTs = [res.tile([128, S], BF16, tag=f'kt{p}', name=f'kt{p}') for p in range(2)]
        v_sb = [res.tile([128, HPC, DH + 1], BF16, tag=f'v{i}', name=f'v{i}')
                for i in range(SK)]
        ones4 = res.tile([128, HPC, 1], F32, tag='ones4')
        nc.vector.memset(ones4, 1.0)

        # ---- projection groups (8 matmuls + eviction), run direct or woven ----
        gid = [0]

        def qk_group(which, p, j4):
            w_r = wq_r if which == 'q' else wk_r
            src = xts if which == 'q' else yts
            dest = QT[p] if which == 'q' else KTs[p]
            js = slice(j4 * JBLK, (j4 + 1) * JBLK)
            hj = slice((j4 % 2) * JBLK, (j4 % 2 + 1) * JBLK)
            cs = slice(p * 128, (p + 1) * 128)
            gid[0] += 1
            ps = ps_pv.tile([128, JBLK], F32, tag='pv', name=f'g{gid[0]}')
            items = []
            for k in range(KT_H):
                def mm(k=k):
                    nc.tensor.matmul(ps, w_r[:, k, cs], src[k][j4 // 2][:, hj],
                                     start=(k == 0), stop=(k == KT_H - 1))
                items.append(mm)

            def fin():
                nc.vector.tensor_copy(dest[:, js], ps)
            items.append(fin)
            return items

        def v_group(j4, m):
            sub = j4 * 4 + m
            hj0 = (j4 % 2) * JBLK + m * 128
            gid[0] += 1
            ps = ps_pv.tile([128, JBLK], F32, tag='pv', name=f'g{gid[0]}')
            items = []
            for k in range(KT_H):
                def mm(k=k):
                    nc.tensor.matmul(ps[:, 0:C],
                                     yts[k][j4 // 2][:, hj0:hj0 + 128],
                                     wv_r[:, k, :],
                                     start=(k == 0), stop=(k == KT_H - 1))
                items.append(mm)

            def fin():
                nc.vector.tensor_scalar_mul(
                    v_sb[sub][:, :, 0:DH],
                    ps[:, 0:C].rearrange('p (h c) -> p h c', h=HPC),
                    eb[:, sub:sub + 1])
                nc.gpsimd.tensor_scalar_mul(v_sb[sub][:, :, DH:DH + 1], ones4,
                                            eb[:, sub:sub + 1])
            items.append(fin)
            return items

        # prefix part 1: the two groups feeding the first QK
        for grp in ([qk_group('k', 0, 0)] + [qk_group('q', 0, 0)]):
            for it in grp:
                it()

        # woven into the attention k-loop under consumption deadlines:
        # dl = the step whose emission (QK lookahead / PV) first needs the
        # group's output.  Service runs at the top of each step, emitting
        # every overdue item plus a small budget of future ones, so writers
        # are always emitted before their readers (Tile's semaphore
        # assignment relies on program order matching dataflow order).
        weave = []
        _seq = [0]
        def _push(dl, groups):
            for grp in groups:
                for it in grp:
                    _seq[0] += 1
                    bisect.insort(weave, (dl, _seq[0], it))
        _push(2, [qk_group('k', 0, 1)])
        _push(4, [v_group(1, 0)]); _push(5, [v_group(1, 1)])
        _push(6, [v_group(1, 2), qk_group('k', 0, 2)])
        _push(7, [v_group(1, 3)])
        _push(8, [v_group(2, 0)]); _push(9, [v_group(2, 1)])
        _push(10, [qk_group('k', 0, 3), v_group(2, 2)])
        _push(11, [v_group(2, 3)])
        _push(12, [v_group(3, 0)]); _push(13, [v_group(3, 1)])
        _push(14, [qk_group('k', 1, 0), qk_group('q', 1, 0), v_group(3, 2)])
        _push(15, [v_group(3, 3)])
        _push(18, [qk_group('k', 1, 1)])
        _push(22, [qk_group('k', 1, 2)])
        _push(26, [qk_group('k', 1, 3)])
        _push(30, [qk_group('q', 0, 1)])
        _push(46, [qk_group('q', 1, 1)])
        _push(62, [qk_group('q', 0, 2)])
        _push(78, [qk_group('q', 1, 2)])
        _push(94, [qk_group('q', 0, 3)])
        _push(110, [qk_group('q', 1, 3)])

        def weave_service(step, budget=2):
            n = 0
            while weave and (weave[0][0] <= step or n < budget):
                _, _, it = weave.pop(0)
                it()
                n += 1

        # ---- attention + output projection ----
        def out_groups(J, ctx_tiles, queues):
            groups = []
            for i, (m, n) in enumerate([(m, n) for m in range(4) for n in range(2)]):
                def grp(m=m, n=n, i=i):
                    ms = slice(m * 128, (m + 1) * 128)
                    ns = slice(n * JBLK, (n + 1) * JBLK)
                    pso = ps_pv.tile([128, JBLK], F32, tag='pv', name=f'o{J}_{m}_{n}')
                    for p in range(2):
                        nc.tensor.matmul(pso, ctx_tiles[p][:, ms], wo_r[:, p, ns],
                                         start=(p == 0), stop=(p == 1))
                    ob = outpool.tile([128, JBLK], F32, tag='ob')
                    nc.vector.tensor_copy(ob, pso)
                    eng = queues[i % len(queues)]
                    eng.dma_start(out=out.ap()[J * JBLK + m * 128:
                                               J * JBLK + (m + 1) * 128, ns],
                                  in_=ob)
                groups.append(grp)
            return groups

        pairs = [(J, p) for J in range(NJ) for p in range(2)]
        psl_q = deque()

        def emit_qk(pidx, kk):
            if pidx >= len(pairs):
                return
            J, p = pairs[pidx]
            js = slice(J * JBLK, (J + 1) * JBLK)
            kks = slice(kk * 128, (kk + 1) * 128)
            psl = ps_qk.tile([128, 2 * JBLK], F32, tag='qk',
                             name=f'psl{pidx}_{kk}')
            nc.tensor.matmul(psl[:, 0:JBLK],
                             KTs[p][0:64, kks], QT[p][0:64, js],
                             start=True, stop=True, tile_position=(0, 0))
            nc.tensor.matmul(psl[:, JBLK:2 * JBLK],
                             KTs[p][64:128, kks], QT[p][64:128, js],
                             start=True, stop=True, tile_position=(64, 0))
            psl_q.append(psl)

        emit_qk(0, 0)
        emit_qk(0, 1)
        # prefix part 2: V blocks for kk=0..3 (consumed after the first exp)
        for grp in [v_group(0, m) for m in range(4)]:
            for it in grp:
                it()
        for J in range(NJ):
            ctx_tiles = []
            for p in range(2):
                pidx = J * 2 + p
                pv0 = ps_pv.tile([128, JBLK], F32, tag='pv')
                pv1 = ps_pv.tile([128, JBLK], F32, tag='pv')
                for kk in range(SK):
                    step = pidx * SK + kk
                    # service the weave FIRST so every producer is emitted
                    # before the QK lookahead / PV that consumes it
                    weave_service(step, budget=3 if pidx < 3 else 2)
                    # QK two steps ahead, crossing pair boundaries so the
                    # next pair's logits are queued before this pair's tail
                    if kk + 2 < SK:
                        emit_qk(pidx, kk + 2)
                    else:
                        emit_qk(pidx + 1, kk + 2 - SK)
                    psl = psl_q.popleft()
                    ex = expool.tile([128, 2 * JBLK], BF16, tag='ex')
                    nc.scalar.activation(ex, psl, mybir.ActivationFunctionType.Exp)
                    for hh, pv in enumerate((pv0, pv1)):
                        hcol = 2 * p + hh
                        nc.tensor.matmul(
                            pv[0:DH + 1, :],
                            v_sb[kk][:, hcol, :],
                            ex[:, hh * JBLK:(hh + 1) * JBLK],
                            start=(kk == 0), stop=(kk == SK - 1))
                # normalize: ctxT[d, q] * (1/denom[q]) via partition broadcast
                ct = ctxpool.tile([128, JBLK], BF16, tag=f'ctx{p}')
                stage = []
                for hh, pv in enumerate((pv0, pv1)):
                    rawct = small.tile([128, JBLK], F32, tag='rawct')
                    nc.vector.tensor_copy(rawct[0:DH + 1, :], pv[0:DH + 1, :])
                    rec = small.tile([128, JBLK], F32, tag='rec')
                    nc.vector.reciprocal_approx_fast(rec[0:DH + 1, :],
                                                     rawct[0:DH + 1, :])
                    bcs = small.tile([128, JBLK], F32, tag='bcs')
                    nq = nc.scalar if pidx >= 6 else nc.sync
                    nq.dma_start(out=bcs[0:1, :], in_=rec[DH:DH + 1, :])
                    bc = small.tile([128, JBLK], F32, tag='bc')
                    nc.gpsimd.partition_broadcast(bc[0:DH, :], bcs[0:1, :])
                    stage.append((rawct, bc))
                for hh, (rawct, bc) in enumerate(stage):
                    if hh == 0:
                        nc.vector.tensor_mul(ct[0:DH, :], rawct[0:DH, :], bc[0:DH, :])
                    else:
                        tmp = small.tile([128, JBLK], BF16, tag='tmp')
                        nc.vector.tensor_mul(tmp[0:DH, :], rawct[0:DH, :], bc[0:DH, :])
                        nq = nc.scalar if pidx >= 6 else nc.sync
                        nq.dma_start(out=ct[DH:128, :], in_=tmp[0:DH, :])
                ctx_tiles.append(ct)
            if J < NJ - 1:
                # out-proj woven into later steps; must fully emit before
                # ct(J+2) overwrites this J's ctx tiles (~step 32*J+79)
                _push(min(32 * J + 64, 120),
                      [[g] for g in out_groups(J, ctx_tiles, [nc.sync])])
            else:
                weave_service(10 ** 9, budget=len(weave))
                for g in out_groups(J, ctx_tiles, [nc.sync, nc.scalar]):
                    g()

    nc.compile()
    return nc


def _get_nc():
    if 'nc' not in _CACHE:
        _CACHE['nc'] = _build()
    return _CACHE['nc']


def shard_inputs(x, y, bias, wq, wk, wv, wo):
    """Build the 8 per-core input maps from full inputs."""
    scale = (H // NH) ** -0.5
    wqs = (wq * scale).astype(np.float32)
    bf = ml_dtypes.bfloat16
    in_maps = []
    for c in range(NCORES):
        b = c // (NCORES // B)
        g = c % (NCORES // B)
        cols = slice(g * C, (g + 1) * C)
        eb = np.exp(bias[b, 0, 0, :].astype(np.float64)).astype(np.float32)
        in_maps.append({
            'xT': np.ascontiguousarray(x[b].T.astype(bf)),
            'yT': np.ascontiguousarray(y[b].T.astype(bf)),
            'wq': np.ascontiguousarray(wqs[:, cols].astype(bf)),
            'wk': np.ascontiguousarray(wk[:, cols].astype(bf)),
            'wv': np.ascontiguousarray(wv[:, cols].astype(bf)),
            'wo': np.ascontiguousarray(wo[cols, :].astype(bf)),
            'ebias': np.ascontiguousarray(eb.reshape(SK, 128).T),
        })
    return in_maps


def kernel(x, y, bias, wq, wk, wv, wo, _trace=False):
    x, y, bias = np.asarray(x), np.asarray(y), np.asarray(bias)
    wq, wk, wv, wo = (np.asarray(t) for t in (wq, wk, wv, wo))
    nc = _get_nc()
    in_maps = shard_inputs(x, y, bias, wq, wk, wv, wo)
    kw = {}
    if _trace:
        kw = dict(trace=True, stitch_traces=False)
    res = bass_utils.run_bass_kernel_spmd(nc, in_maps, core_ids=list(range(NCORES)), **kw)
    full = np.zeros((B, S, H), dtype=np.float64)
    for c in range(NCORES):
        full[c // (NCORES // B)] += res.results[c]['out'].astype(np.float64)
    if _trace:
        _CACHE['last_results'] = res
    return full.astype(np.float32)
